# revision 16
# baseline (speedup 1.0000x reference)
"""ConvAttention (GroupNorm + channel attention + residual) on 8 Trainium2
NeuronCores, data-parallel over batch (B=8 -> 1 item/core).

GroupNorm is folded into the attention algebra; g is never materialized:

  g = D x + beta 1^T          (D = diag(a), per-channel affine from stats)
  scores = Wq D Gx D Wk^T + rank-1 corrections (qp/qb/kp/kb rows)
  attn^T = x^T (D Wv^T probs^T) + 1 (x) pv,  pv = (Wv beta + bv)^T probs^T

v3 (measured-cost-model rewrite of v2):
  - PE streams rhs at ~2B/cycle/partition only when the access is
    contiguous; strided pair reads run at ~half rate.  So the Gram is
    computed FULL-WIDTH (every block row spans all 512 columns,
    contiguous 1024B DoubleRow reads) instead of upper-triangle with
    strided slices + PE transposes: fewer ns despite 1.6x the MACs,
    and the transpose/evac machinery disappears.
  - GN stats from the Gram: channel row sums sx via F=1 matmuls
    against a memset ones-pair tile; E[x^2] from the Gram diagonal
    (ident-mult + reduce on DVE).  No bn_stats pass.
  - a is folded into wq/wk rows (awq/awk); Gx is evacuated raw.
  - pv: one Radd materialization, folded per-tile into the xr residual
    tiles on DVE.  Even tiles evacuate as at+sl on DVE; odd tiles get
    the residual via a PE identity matmul and a pure Act cast.
  - All matmul rhs operands are contiguous per-partition runs.
"""
import sys

if "/opt/trn_rl_repo" not in sys.path:
    sys.path.insert(0, "/opt/trn_rl_repo")

from contextlib import ExitStack

import ml_dtypes
import numpy as np

import concourse.bass as bass
import concourse.tile as tile
from concourse import bacc, mybir
from concourse import bass_utils
from concourse.masks import make_identity

BF16 = ml_dtypes.bfloat16
F8NP = ml_dtypes.float8_e4m3fn
bf = mybir.dt.bfloat16
f32 = mybir.dt.float32
f8 = mybir.dt.float8e4

B, C, H, W = 8, 512, 64, 64
N = H * W            # 4096 spatial tokens
GROUPS = 32
GS = C // GROUPS     # 16 channels per group
EPS = 1e-6
ALPHA = float(C) ** -0.5
P = 128
CT = C // P          # 4 channel tiles
NT = N // P          # 32 spatial tiles
NPAIR = NT // 2      # 16 DoubleRow token-pair tiles

AF = mybir.ActivationFunctionType
AX = mybir.AxisListType
OP = mybir.AluOpType
PM = mybir.MatmulPerfMode


def _build_program():
    nc = bacc.Bacc("TRN2", target_bir_lowering=False, debug=False, num_devices=B)

    xt2_d = nc.dram_tensor("xt2", (NPAIR * P, 2 * C), f8, kind="ExternalInput").ap()
    xf2_d = nc.dram_tensor("xf2", (2 * P, 2 * N), f8, kind="ExternalInput").ap()
    xr_d = nc.dram_tensor("xr", (N, C), bf, kind="ExternalInput").ap()
    wqT_d = nc.dram_tensor("wqT", (C, C), bf, kind="ExternalInput").ap()
    wkT_d = nc.dram_tensor("wkT", (C, C), bf, kind="ExternalInput").ap()
    wv_d = nc.dram_tensor("wv", (C, C), bf, kind="ExternalInput").ap()
    wvT_d = nc.dram_tensor("wvT", (C, C), bf, kind="ExternalInput").ap()
    # smA f32 [128, 40]: gnw (cols 0:4), gnb (4:8), ind16 (8:40)
    smA_d = nc.dram_tensor("smA", (P, 40), f32, kind="ExternalInput").ap()
    smB_d = nc.dram_tensor("smB", (8, P), f32, kind="ExternalInput").ap()
    # smC bf16 [2, 8]: lq2 (cols 0:4), lk2 (4:8)
    smC_d = nc.dram_tensor("smC", (2, 8), bf, kind="ExternalInput").ap()
    # smD bf16 [1, 1040]: bq (0:512), bk (512:1024), lq1 (1024:1028), lk1 (1028:1032)
    smD_d = nc.dram_tensor("smD", (1, 1040), bf, kind="ExternalInput").ap()
    smE_d = nc.dram_tensor("smE", (1, C), f32, kind="ExternalInput").ap()
    out_d = nc.dram_tensor("out", (N, C), bf, kind="ExternalOutput").ap()

    with tile.TileContext(nc) as tc, ExitStack() as ctx:
        consts = ctx.enter_context(tc.tile_pool(name="consts", bufs=1))
        pxt = ctx.enter_context(tc.tile_pool(name="pxt", bufs=1))
        pmats = ctx.enter_context(tc.tile_pool(name="pmats", bufs=1))
        psmall = ctx.enter_context(tc.tile_pool(name="psmall", bufs=4))
        presid = ctx.enter_context(tc.tile_pool(name="presid", bufs=1))
        pout = ctx.enter_context(tc.tile_pool(name="pout", bufs=1))
        ps_big = ctx.enter_context(tc.tile_pool(name="ps_big", bufs=3, space="PSUM"))
        ps_gctx = ExitStack()
        ps_gram = ps_gctx.enter_context(tc.tile_pool(name="ps_gram", bufs=1, space="PSUM"))

        # ---------------- DMA plan (issued before anything else) ----------
        # sync+gpsimd rings carry the loads; scalar the small constants.
        xt2_sb = pxt.tile([P, NPAIR, 2 * C], f8, tag="xt2")
        xt2v = xt2_d.rearrange("(j p) f -> p j f", p=P)
        xt2_eng = [nc.sync, nc.gpsimd, nc.sync, nc.gpsimd,
                   nc.sync, nc.gpsimd, nc.sync, nc.gpsimd]
        for r in range(8):
            xt2_eng[r].dma_start(xt2_sb[:, 2 * r:2 * r + 2, :],
                                 xt2v[:, 2 * r:2 * r + 2, :])
        wq_sb = consts.tile([P, CT, C], bf, tag="wq")
        nc.sync.dma_start(wq_sb, wqT_d.rearrange("(t p) c -> p t c", p=P))
        wk_sb = consts.tile([P, CT, C], bf, tag="wk")
        nc.gpsimd.dma_start(wk_sb, wkT_d.rearrange("(t p) c -> p t c", p=P))
        wv_sb = consts.tile([P, CT, C], bf, tag="wv")
        nc.sync.dma_start(wv_sb, wv_d.rearrange("(t p) c -> p t c", p=P))
        wvT_sb = consts.tile([P, CT, C], bf, tag="wvT")
        nc.gpsimd.dma_start(wvT_sb, wvT_d.rearrange("(t p) c -> p t c", p=P))
        xf2_sb = [pxt.tile([P, 2, N], f8, tag=f"xf{t}", name=f"xf2sb{t}")
                  for t in range(2)]
        nc.sync.dma_start(xf2_sb[0], xf2_d[0:P, :].rearrange("p (i n) -> p i n", i=2))
        nc.gpsimd.dma_start(xf2_sb[1], xf2_d[P:2 * P, :].rearrange("p (i n) -> p i n", i=2))
        rb_sb = []
        rb_eng = [nc.sync, nc.gpsimd, nc.sync, nc.gpsimd,
                  nc.sync, nc.gpsimd, nc.sync, nc.gpsimd]
        for g in range(8):
            rb = presid.tile([P, 4, C], bf, tag=f"rb{g}", name=f"rbsb{g}")
            rb_eng[g].dma_start(rb, xr_d[g * 4 * P:(g + 1) * 4 * P, :]
                                .rearrange("(q p) f -> p q f", p=P))
            rb_sb.append(rb)
        # consolidated small constants on the scalar ring
        smA = consts.tile([P, 40], f32, tag="smA")
        nc.scalar.dma_start(smA, smA_d)
        smB = consts.tile([8, P], f32, tag="smB")
        nc.scalar.dma_start(smB, smB_d)
        smC = consts.tile([2, 8], bf, tag="smC")
        nc.scalar.dma_start(smC, smC_d)
        smD = consts.tile([1, 1040], bf, tag="smD")
        nc.scalar.dma_start(smD, smD_d)
        smE = consts.tile([1, C], f32, tag="smE")
        nc.scalar.dma_start(smE, smE_d)

        # ---------------- on-chip constants (no DMA) ----------------
        dum11 = consts.tile([1, 1], f32, tag="dum11")
        nc.vector.memset(dum11, 1.0)
        # warm the sqrt activation table (scalar engine op #1)
        dsq = psmall.tile([1, 1], f32, tag="dsq")
        nc.scalar.activation(dsq, dum11, AF.Sqrt, bias=0.0, scale=1.0)
        ident = consts.tile([P, P], bf, tag="ident")
        make_identity(nc, ident)
        identN = consts.tile([P, P], bf, tag="identN")
        nc.vector.tensor_scalar(identN, ident, 1.0 / N, None, op0=OP.mult)
        eps8 = consts.tile([8, 1], f32, tag="eps8")
        nc.vector.memset(eps8, EPS)
        ones1 = consts.tile([1, P], bf, tag="ones1")
        nc.vector.memset(ones1, 1.0)
        one11 = consts.tile([1, 1], bf, tag="one11")
        nc.vector.memset(one11, 1.0)
        ones2 = consts.tile([P, 2, 16], f8, tag="ones2")
        nc.vector.memset(ones2, 1.0)

        gnw_c = [smA[:, t:t + 1] for t in range(CT)]
        gnb_c = [smA[:, 4 + t:5 + t] for t in range(CT)]
        i16_c = [smA[:, 8 + 8 * t:16 + 8 * t] for t in range(CT)]
        iT_sb = smB
        lq2 = smC[:, 0:4]
        lk2 = smC[:, 4:8]
        bqr = smD[:, 0:C]
        bkr = smD[:, C:2 * C]
        lq1 = smD[0:1, 2 * C:2 * C + 4]
        lk1 = smD[0:1, 2 * C + 4:2 * C + 8]
        bvr = smE

        # ---------------- Gram (full-width, fp8 DoubleRow) + sx -----------
        # Full rows: contiguous 1024B rhs per pass beats the symmetric
        # upper-triangle whose strided slices halve the PE feed rate.
        # sx rides the same stationaries against a ones-pair tile; the
        # four sx columns share one PSUM zero-region, so only the first
        # may start=True (the start zeroes the whole 2KB bank).
        G_ps = [ps_gram.tile([P, C], f32, tag=f"G{i}", name=f"Gps{i}")
                for i in range(CT)]
        sx_ps = ps_gram.tile([P, 4], f32, tag="sx")
        for j in range(NPAIR):
            xt2j = xt2_sb[:, j, :].rearrange("p (i c) -> p i c", i=2)
            for io in range(CT):
                nc.tensor.matmul(G_ps[io],
                                 lhsT=xt2j[:, :, io * P:(io + 1) * P],
                                 rhs=xt2j,
                                 start=(j == 0), stop=(j == NPAIR - 1),
                                 perf_mode=PM.DoubleRow)
                nc.tensor.matmul(sx_ps[:, io:io + 1],
                                 lhsT=xt2j[:, :, io * P:(io + 1) * P],
                                 rhs=ones2[:, :, 0:1],
                                 start=(j == 0 and io == 0),
                                 stop=(j == NPAIR - 1 and io == CT - 1),
                                 perf_mode=PM.DoubleRow, skip_group_check=True)

        # ---------------- GN coefficients from the Gram -------------------
        # st2[io] = [sx/N, diag/N] = [mean_c, E[x^2]_c]
        st2_sb = []
        for io in range(CT):
            st2 = psmall.tile([P, 2], f32, tag=f"st2{io}", bufs=1, name=f"st2_{io}")
            nc.vector.tensor_scalar(st2[:, 0:1], sx_ps[:, io:io + 1],
                                    1.0 / N, None, op0=OP.mult)
            junk = psmall.tile([P, P], f32, tag="junk", bufs=2)
            nc.vector.tensor_tensor(junk, G_ps[io][:, io * P:(io + 1) * P],
                                    identN, OP.mult)
            nc.vector.reduce_sum(st2[:, 1:2], junk, axis=AX.X)
            st2_sb.append(st2)

        # stats chains: gst matmuls first so the per-block chains pipeline
        # through DVE/Act; raw Gx evacuations interleave on Act.
        gr2_sb, Gx_sb = [], []
        for io in range(CT):
            gst = ps_big.tile([8, 2], f32, tag="big")
            nc.tensor.matmul(gst, lhsT=i16_c[io], rhs=st2_sb[io],
                             start=True, stop=True)
            Gx = pmats.tile([P, C], bf, tag=f"Gx{io}", name=f"Gxsb{io}")
            nc.scalar.copy(Gx, G_ps[io])
            Gx_sb.append(Gx)
            gtmp = psmall.tile([8, 1], f32, tag="gtmp")
            nc.vector.tensor_scalar(gtmp, gst[:, 0:1], gst[:, 0:1], None,
                                    op0=OP.mult)
            gvar = psmall.tile([8, 1], f32, tag="gvar")
            nc.vector.tensor_tensor(gvar, gst[:, 1:2], gtmp, OP.subtract)
            gsd = psmall.tile([8, 1], f32, tag="gsd")
            nc.scalar.activation(gsd, gvar, AF.Sqrt, bias=eps8, scale=1.0)
            grs = psmall.tile([8, 1], f32, tag="grs")
            nc.vector.reciprocal(grs, gsd)
            gr2 = psmall.tile([8, 2], f32, tag=f"gr2{io}", bufs=1, name=f"gr2_{io}")
            nc.vector.tensor_copy(gr2[:, 0:1], gst[:, 0:1])
            nc.vector.tensor_copy(gr2[:, 1:2], grs)
            gr2_sb.append(gr2)
        a_sb, beta_sb, pb_sb = [], [], []
        awq_sb, awk_sb = [], []
        for io in range(CT):
            bc = ps_big.tile([P, 2], f32, tag="big")
            nc.tensor.matmul(bc, lhsT=iT_sb, rhs=gr2_sb[io], start=True, stop=True)
            a_col = psmall.tile([P, 1], f32, tag=f"a{io}", bufs=1, name=f"acol{io}")
            nc.vector.tensor_tensor(a_col, gnw_c[io], bc[:, 1:2], OP.mult)
            tmp = psmall.tile([P, 1], f32, tag="tmp")
            nc.vector.tensor_tensor(tmp, bc[:, 0:1], a_col, OP.mult)
            b_col = psmall.tile([P, 1], f32, tag=f"b{io}", bufs=1, name=f"bcol{io}")
            nc.vector.tensor_tensor(b_col, gnb_c[io], tmp, OP.subtract)
            a_sb.append(a_col)
            beta_col = psmall.tile([P, 1], bf, tag=f"bb{io}", bufs=1, name=f"betac{io}")
            nc.vector.tensor_copy(beta_col, b_col)
            beta_sb.append(beta_col)
            pb = psmall.tile([P, 2], bf, tag=f"pb{io}", bufs=1, name=f"pbc{io}")
            nc.vector.tensor_scalar(pb[:, 0:1], st2_sb[io][:, 0:1], a_col,
                                    float(N), op0=OP.mult, op1=OP.mult)
            nc.vector.tensor_copy(pb[:, 1:2], b_col)
            pb_sb.append(pb)
            # awq/awk: fold a into the wq/wk rows (gates A^T / scores)
            awq_t = pmats.tile([P, C], bf, tag=f"awq{io}", name=f"awqsb{io}")
            nc.vector.tensor_scalar(awq_t, wq_sb[:, io, :], a_col, None, op0=OP.mult)
            awq_sb.append(awq_t)
            awk_t = pmats.tile([P, C], bf, tag=f"awk{io}", name=f"awksb{io}")
            nc.vector.tensor_scalar(awk_t, wk_sb[:, io, :], a_col, None, op0=OP.mult)
            awk_sb.append(awk_t)

        # dummy exp right after the last sqrt: the Act table set switch
        # (1283ns) hides under the A^T/scores phase
        dex = psmall.tile([1, 1], f32, tag="dex")
        nc.scalar.activation(dex, dum11, AF.Exp, bias=0.0, scale=1.0)

        # ---------------- A^T = Gx^T awq pipelined into scores ------------
        # Full Gram means every lhsT block is a direct slice; A^T(jt-1)
        # matmuls are emitted before the scores matmuls of jt so the PE
        # never waits on an A^T evacuation.
        ps_gctx.close()
        ps_tctx = ExitStack()
        ps_tr = ps_tctx.enter_context(tc.tile_pool(name="ps_tr", bufs=1, space="PSUM"))
        ps_qctx = ExitStack()
        ps_quad = ps_qctx.enter_context(tc.tile_pool(name="ps_quad", bufs=1, space="PSUM"))
        scp = [ps_quad.tile([P, C], f32, tag=f"q{ct}", name=f"scq{ct}")
               for ct in range(CT)]
        AT_sb = [None] * CT
        at_eng = [nc.scalar, nc.vector, nc.scalar, nc.vector]
        Ap_sb = [None] * CT

        def emit_AT(jt):
            Ap = ps_big.tile([P, C], f32, tag="big")
            for it in range(CT):
                nc.tensor.matmul(Ap, lhsT=Gx_sb[it][:, jt * P:(jt + 1) * P],
                                 rhs=awq_sb[it],
                                 start=(it == 0), stop=(it == CT - 1))
            Ap_sb[jt] = Ap

        emit_AT(3)
        for idx, jt in enumerate(range(CT - 1, -1, -1)):
            AT_t = pmats.tile([P, C], bf, tag=f"AT{jt}", name=f"ATsb{jt}")
            if at_eng[idx] is nc.scalar:
                nc.scalar.copy(AT_t, Ap_sb[jt])
            else:
                nc.vector.tensor_copy(AT_t, Ap_sb[jt])
            AT_sb[jt] = AT_t
            if jt > 0:
                emit_AT(jt - 1)
            for ct in range(CT):
                nc.tensor.matmul(scp[ct], lhsT=AT_t[:, ct * P:(ct + 1) * P],
                                 rhs=awk_sb[jt], start=(idx == 0), stop=False)

        # rank-1 row machinery (PE-tiny; pb/beta-gated)
        qrows_p = ps_big.tile([2, C], f32, tag="big")
        for ci in range(CT):
            nc.tensor.matmul(qrows_p, lhsT=pb_sb[ci], rhs=wq_sb[:, ci, :],
                             start=(ci == 0), stop=(ci == CT - 1))
        qr2 = pmats.tile([2, C], bf, tag="qr2")
        nc.vector.tensor_copy(qr2, qrows_p)
        krows_p = ps_big.tile([2, C], f32, tag="big")
        for ci in range(CT):
            nc.tensor.matmul(krows_p, lhsT=pb_sb[ci], rhs=wk_sb[:, ci, :],
                             start=(ci == 0), stop=(ci == CT - 1))
        kr2 = pmats.tile([2, C], bf, tag="kr2")
        nc.vector.tensor_copy(kr2, krows_p)
        rq_p = ps_big.tile([4, C], f32, tag="big")
        nc.tensor.matmul(rq_p, lhsT=lq2, rhs=qr2, start=True, stop=False)
        nc.tensor.matmul(rq_p, lhsT=lq1, rhs=bqr, start=False, stop=True)
        rows_q = pmats.tile([4, C], bf, tag="rows_q")
        nc.vector.tensor_copy(rows_q, rq_p)
        rk_p = ps_big.tile([4, C], f32, tag="big")
        nc.tensor.matmul(rk_p, lhsT=lk2, rhs=kr2, start=True, stop=False)
        nc.tensor.matmul(rk_p, lhsT=lk1, rhs=bkr, start=False, stop=True)
        rows_k = pmats.tile([4, C], bf, tag="rows_k")
        nc.vector.tensor_copy(rows_k, rk_p)

        # ---------------- softmax (no max subtraction) + probs^T ----------
        # scores are O(+-60); alpha*s stays well inside exp's f32 range.
        pr_sb, prT = [], pmats.tile([P, CT, C], bf, tag="prT")
        for ct in range(CT):
            nc.tensor.matmul(scp[ct], lhsT=rows_q[:, ct * P:(ct + 1) * P],
                             rhs=rows_k, start=False, stop=True)
            se = psmall.tile([P, 1], f32, tag=f"se{ct}", bufs=1, name=f"sec{ct}")
            pr_t = pmats.tile([P, C], bf, tag=f"pr{ct}", name=f"prsb{ct}")
            nc.scalar.activation(pr_t, scp[ct], AF.Exp, bias=0.0, scale=ALPHA,
                                 accum_out=se)
            ri = psmall.tile([P, 1], f32, tag="ri")
            nc.vector.reciprocal(ri, se)
            nc.vector.tensor_scalar_mul(pr_t, pr_t, ri)
            pr_sb.append(pr_t)
        ps_qctx.close()

        for ct in range(CT):
            trp = ps_tr.tile([P, C], bf, tag="tr")
            for dt in range(CT):
                nc.tensor.transpose(trp[:, dt * P:(dt + 1) * P],
                                    pr_sb[ct][:, dt * P:(dt + 1) * P], ident)
            nc.vector.tensor_copy(prT[:, :, ct * P:(ct + 1) * P],
                                  trp.rearrange("p (a b) -> p a b", a=CT))

        # vb columns (Wv beta + bv per-channel) while the transposes run
        vrow_p = ps_big.tile([1, C], f32, tag="big")
        for ci in range(CT):
            nc.tensor.matmul(vrow_p, lhsT=beta_sb[ci], rhs=wvT_sb[:, ci, :],
                             start=(ci == 0), stop=(ci == CT - 1))
        vbrow = pmats.tile([1, C], bf, tag="vbrow")
        nc.vector.tensor_tensor(vbrow, vrow_p, bvr, OP.add)
        vb_cols = []
        for dt in range(CT):
            cp = ps_big.tile([P, 1], f32, tag="big")
            nc.tensor.matmul(cp, lhsT=vbrow[0:1, dt * P:(dt + 1) * P], rhs=one11,
                             start=True, stop=True)
            vb_c = psmall.tile([P, 1], bf, tag=f"vb{dt}", bufs=1, name=f"vbc{dt}")
            nc.vector.tensor_copy(vb_c, cp)
            vb_cols.append(vb_c)

        # ---------------- M^T (F=512 accumulation) + MT2 + pv -------------
        ps_qctx2 = ExitStack()
        ps_m = ps_qctx2.enter_context(tc.tile_pool(name="ps_m", bufs=1, space="PSUM"))
        Mp = [ps_m.tile([P, C], f32, tag=f"m{it}", name=f"Mpq{it}")
              for it in range(CT)]
        for it in range(CT):
            for dt in range(CT):
                nc.tensor.matmul(Mp[it], lhsT=wv_sb[:, dt, it * P:(it + 1) * P],
                                 rhs=prT[:, dt, :],
                                 start=(dt == 0), stop=(dt == CT - 1))
        # MT2 evacs gate the attn DR matmuls -> emitted before pvb on Act
        MT2_sb = [pmats.tile([P, 2, C], f8, tag=f"MT2{t}", name=f"MT2sb{t}")
                  for t in range(2)]
        for it in range(CT):
            nc.scalar.activation(MT2_sb[it // 2][:, it % 2, :], Mp[it],
                                 AF.Copy, bias=0.0, scale=a_sb[it])
        pvp = ps_big.tile([1, C], f32, tag="big")
        for dt in range(CT):
            nc.tensor.matmul(pvp, lhsT=vb_cols[dt], rhs=prT[:, dt, :],
                             start=(dt == 0), stop=(dt == CT - 1))
        pvb = pmats.tile([1, C], bf, tag="pvb")
        nc.scalar.copy(pvb, pvp)
        # Radd = 1 (x) pv, materialized once; folded per-tile into rb
        Rp = ps_big.tile([P, C], f32, tag="big")
        nc.tensor.matmul(Rp, lhsT=ones1, rhs=pvb, start=True, stop=True)
        Radd = pmats.tile([P, C], bf, tag="Radd")
        nc.scalar.copy(Radd, Rp)

        # ---------------- attn^T + residual + store ----------------
        # Per tile: fold Radd into the xr slice (DVE, in-place), then
        # even tiles evacuate as at+sl on DVE; odd tiles add sl via a PE
        # identity matmul and evacuate with a pure Act cast.
        ps_qctx2.close()
        ps_tctx.close()
        ps_att = ctx.enter_context(tc.tile_pool(name="ps_att", bufs=5, space="PSUM"))
        store_eng = [nc.sync, nc.scalar, nc.gpsimd]
        osb_g = None
        for nt in range(NT):
            g, q = nt // 4, nt % 4
            if q == 0:
                osb_g = pout.tile([P, 4, C], bf, tag=f"osb{g}", name=f"osb_{g}")
            sl = rb_sb[g][:, q, :]
            nc.vector.tensor_tensor(sl, sl, Radd, OP.add)
            at = ps_att.tile([P, C], f32, tag="att", name=f"at{nt}")
            for t in range(2):
                nc.tensor.matmul(at, lhsT=xf2_sb[t][:, :, nt * P:(nt + 1) * P],
                                 rhs=MT2_sb[t], start=(t == 0),
                                 stop=(t == 1 and nt % 2 == 0),
                                 perf_mode=PM.DoubleRow, skip_group_check=True)
            if nt % 2 == 0:
                nc.vector.tensor_tensor(osb_g[:, q, :], at, sl, OP.add)
            else:
                nc.tensor.matmul(at, lhsT=ident, rhs=sl,
                                 start=False, stop=True, skip_group_check=True)
                nc.scalar.copy(osb_g[:, q, :], at)
            store_eng[nt % 3].dma_start(out_d[nt * P:(nt + 1) * P, :],
                                        osb_g[:, q, :])

    nc.compile()
    return nc


_NC = None


def _get_program():
    global _NC
    if _NC is None:
        _NC = _build_program()
    return _NC


def _stage_inputs(x, gn_w, gn_b, wq, bq, wk, bk, wv, bv):
    """Host-side sharding + layout/dtype staging (per-core input maps)."""
    x = np.asarray(x, dtype=np.float32).reshape(B, C, N)
    ind16 = np.zeros((C, 8), np.float32)
    for c in range(C):
        ind16[c, (c % P) // GS] = 1.0 / GS
    indT = np.zeros((8, P), np.float32)
    for p in range(P):
        indT[p // GS, p] = 1.0
    smA = np.zeros((P, 40), np.float32)
    smA[:, 0:4] = np.asarray(gn_w, np.float32).reshape(4, P).T
    smA[:, 4:8] = np.asarray(gn_b, np.float32).reshape(4, P).T
    smA[:, 8:40] = ind16.reshape(4, P, 8).transpose(1, 0, 2).reshape(P, 32)
    smC = np.zeros((2, 8), np.float32)
    smC[:, 0:4] = np.array([[1, 0, 1, 0], [0, 1, N, 0]], np.float32)
    smC[:, 4:8] = np.array([[0, 1, 0, 1], [1, N, 0, N]], np.float32)
    smD = np.zeros((1, 1040), np.float32)
    smD[0, 0:C] = np.asarray(bq, np.float32)
    smD[0, C:2 * C] = np.asarray(bk, np.float32)
    smD[0, 2 * C:2 * C + 4] = np.array([0, 0, 0, 1], np.float32)
    smD[0, 2 * C + 4:2 * C + 8] = np.array([0, 0, 1, N], np.float32)
    shared = {
        "wqT": np.ascontiguousarray(np.asarray(wq, np.float32).T).astype(BF16),
        "wkT": np.ascontiguousarray(np.asarray(wk, np.float32).T).astype(BF16),
        "wv": np.ascontiguousarray(np.asarray(wv, np.float32)).astype(BF16),
        "wvT": np.ascontiguousarray(np.asarray(wv, np.float32).T).astype(BF16),
        "smA": smA,
        "smB": indT,
        "smC": smC.astype(BF16),
        "smD": smD.astype(BF16),
        "smE": np.asarray(bv, np.float32).reshape(1, C),
    }
    in_maps = []
    for b in range(B):
        m = dict(shared)
        xb = x[b]
        # x^T, DoubleRow pair-interleaved (tokens n and n+128 share a row)
        xt2 = (xb.T.reshape(NPAIR, 2, P, C).transpose(0, 2, 1, 3)
               .reshape(NPAIR * P, 2 * C)).astype(F8NP)
        m["xt2"] = np.ascontiguousarray(xt2)
        # x (C, N) fp8, channel-pair interleaved for the attention lhsT
        xf2 = (xb.reshape(2, 2, P, N).transpose(0, 2, 1, 3)
               .reshape(2 * P, 2 * N)).astype(F8NP)
        m["xf2"] = np.ascontiguousarray(xf2)
        # x.flat reinterpreted as (N, C) for the residual
        m["xr"] = np.ascontiguousarray(xb.reshape(N, C)).astype(BF16)
        in_maps.append(m)
    return in_maps


def kernel(x, gn_w, gn_b, wq, bq, wk, bk, wv, bv, _trace=False, _tmpdir=None):
    nc = _get_program()
    in_maps = _stage_inputs(x, gn_w, gn_b, wq, bq, wk, bk, wv, bv)
    res = bass_utils.run_bass_kernel_spmd(
        nc, in_maps, core_ids=list(range(B)), trace=_trace, tmpdir=_tmpdir,
    )
    out = np.stack([res.results[b]["out"].reshape(C, H, W) for b in range(B)])
    if _trace:
        kernel._last_results = res
    return out.astype(np.float32)
